# revision 1
# baseline (speedup 1.0000x reference)
"""Contrastive (SimCLR-style) loss on 8 Trainium2 NeuronCores.

Math (matches the reference exactly):
  P = concat(projection1, projection2)            # [8192, 256]
  sim = cos_sim(P_i, P_j); diag masked to -1e9; logits = sim / 0.5
  labels = arange(2B)  -> picks the masked diagonal, so
  loss = -mean_i( logp_ii ),  logp_ii = f32(-2e9 - lse_i),
  lse_i = log(sum_{j != i} exp(2*sim_ij))

Distribution: data-parallel over the 8192 rows.  Each core receives the
full projection matrix (row-major fp32 for norms + pre-transposed bf16
for the matmul operand) plus its own 1024-row block.  On chip it:
  - computes row norms (DVE square+reduce, Newton rsqrt -- no ScalarE),
  - scales the transposed operand by 1/norm (bf16),
  - matmuls its row block against all 8192 columns (bf16, fp32 PSUM),
  - streams exp through ScalarE with fused row-sum accumulation,
  - subtracts the diagonal term and takes log.
Host all-reduces the per-row lse partials and applies the reference's
fp32 arithmetic for the final mean.
"""

import sys

for _p in ("/opt/trn_rl_repo", "/root/.axon_site/_ro/trn_rl_repo"):
    if _p not in sys.path:
        sys.path.append(_p)

import numpy as np

import concourse.bacc as bacc
import concourse.tile as tile
from concourse import mybir
from concourse import bass_utils

F32 = mybir.dt.float32
BF16 = mybir.dt.bfloat16
I32 = mybir.dt.int32
AF = mybir.ActivationFunctionType
ALU = mybir.AluOpType

N_CORES = 8
B = 8192          # total rows (2 * batch)
D = 256           # projection dim
BLK = B // N_CORES        # 1024 rows per core
M_TILES = BLK // 128      # 8 row tiles per core
N_COLS = 512              # matmul free dim (one PSUM bank)
GROUP = 2048              # ACT exp batch (4 PSUM banks) = one column group
N_GROUPS = B // GROUP     # 4
N_PER_GROUP = GROUP // N_COLS  # 4
U = 16                    # consecutive rows per partition in stats loads
RSQRT_MAGIC = 0x5F3759DF


def _newton_rsqrt(nc, pool, out_rn, s):
    """out_rn = 1/sqrt(s), entirely on VectorE (fp32).

    Quake-style bit seed + 2 Newton iterations (~5e-6 rel err).  Keeps
    ScalarE free for exp and avoids sqrt<->exp table reloads.
    """
    p, w = s.shape
    ibits = pool.tile([p, w], I32, name="ibits", tag="rsq_i", bufs=2)
    nc.vector.tensor_scalar(
        out=ibits, in0=s.bitcast(I32), scalar1=1, scalar2=None,
        op0=ALU.arith_shift_right,
    )
    nc.vector.tensor_scalar(
        out=ibits, in0=ibits, scalar1=-1, scalar2=RSQRT_MAGIC,
        op0=ALU.mult, op1=ALU.add,
    )
    y = ibits.bitcast(F32)
    t1 = pool.tile([p, w], F32, name="t1", tag="rsq_t1", bufs=2)
    for _ in range(2):
        nc.vector.tensor_mul(t1, y, y)
        nc.vector.tensor_mul(t1, t1, s)
        nc.vector.tensor_scalar(
            out=t1, in0=t1, scalar1=-0.5, scalar2=1.5,
            op0=ALU.mult, op1=ALU.add,
        )
        nc.vector.tensor_mul(y, y, t1)
    nc.vector.tensor_copy(out_rn, y)


def _emit(tc, p_stats, pt, p_blk, eye_in, lse_out):
    nc = tc.nc

    persist = tc.alloc_tile_pool(name="persist", bufs=1)
    pin = tc.alloc_tile_pool(name="pin", bufs=2)
    work = tc.alloc_tile_pool(name="work", bufs=2)
    dram = tc.alloc_tile_pool(name="dram", bufs=1, space="DRAM")
    epool = tc.alloc_tile_pool(name="epool", bufs=2)

    # Persistent tensors
    qt0 = persist.tile([128, B], BF16, tag="qt0", name="qt0")
    qt1 = persist.tile([128, B], BF16, tag="qt1", name="qt1")
    bt0 = persist.tile([128, BLK], BF16, tag="bt0", name="bt0")
    bt1 = persist.tile([128, BLK], BF16, tag="bt1", name="bt1")
    q_b = persist.tile([128, M_TILES, D], BF16, tag="q_b", name="q_b")
    rn_f = persist.tile([128, 64], F32, tag="rn_f", name="rn_f")
    rn_b = persist.tile([128, M_TILES], F32, tag="rn_b", name="rn_b")
    selfdot = persist.tile([128, M_TILES], F32, tag="selfdot", name="selfdot")
    sums = persist.tile([128, N_GROUPS * M_TILES], F32, tag="sums", name="sums")
    rowsum = persist.tile([128, M_TILES], F32, tag="rowsum", name="rowsum")
    exps = persist.tile([128, M_TILES], F32, tag="exps", name="exps")
    lse = persist.tile([128, M_TILES], F32, tag="lse", name="lse")
    dram_rn = dram.tile([B], F32, tag="dram_rn", name="dram_rn")

    # ---- This core's row block: norms, scale, self-dot, transpose ----
    pb = p_blk.rearrange("(t p) d -> t p d", p=128)    # [8, 128, 256]
    blk = persist.tile([128, M_TILES, D], F32, tag="blk", name="blk")
    eye = persist.tile([128, 128], BF16, tag="eye", name="eye")
    nc.gpsimd.dma_start(out=eye, in_=eye_in)
    for t in range(M_TILES):
        nc.gpsimd.dma_start(out=blk[:, t, :], in_=pb[t])
    sq_b = work.tile([128, M_TILES, D], BF16, name="sq_b", tag="sq_b", bufs=1)
    nc.vector.tensor_mul(sq_b, blk, blk)
    stats_b = work.tile([128, M_TILES], F32, name="stats_b", tag="stats_b", bufs=1)
    nc.vector.tensor_reduce(stats_b, sq_b, axis=mybir.AxisListType.X, op=ALU.add)
    _newton_rsqrt(nc, work, rn_b, stats_b)
    for t in range(M_TILES):
        nc.vector.tensor_scalar_mul(q_b[:, t, :], blk[:, t, :], rn_b[:, t : t + 1])
    sq_b2 = work.tile([128, M_TILES, D], BF16, name="sq_b2", tag="sq_b", bufs=1)
    nc.vector.tensor_mul(sq_b2, q_b, q_b)
    nc.vector.tensor_reduce(selfdot, sq_b2, axis=mybir.AxisListType.X, op=ALU.add)
    # Transpose the block on the (otherwise idle) tensor engine; copy the
    # PSUM results to SBUF on the scalar engine.  This keeps the slow DMA
    # xbar out of the picture and frees the main loop from DMA-queue deps.
    tp_psum = tc.alloc_tile_pool(name="tp_psum", bufs=4, space="PSUM")
    for t in range(M_TILES):
        for half, btk in ((0, bt0), (1, bt1)):
            tp = tp_psum.tile([128, 128], BF16, name="tp")
            nc.tensor.transpose(tp, q_b[:, t, half * 128 : half * 128 + 128], eye)
            nc.scalar.copy(out=btk[:, t * 128 : (t + 1) * 128], in_=tp)
    tp_psum.release()
    psum_pool = tc.alloc_tile_pool(name="psum", bufs=2, space="PSUM")

    # ---- Full-matrix norms + scaled transposed operand, one group at a
    # time (group g covers columns [2048g, 2048(g+1)) = rows with the
    # same indices; the u=16 interleave keeps j-order identity) ----
    # stats load: row j = 2048t + 16p + u  ->  tile t, partition p, slot u
    ps4 = p_stats.rearrange("(t p u) d -> t p (u d)", p=128, u=U)  # [4,128,4096]
    # rn store: dram_rn[2048t + 16p + u] <- rn_small[p, 16t + u]
    rn_store = dram_rn.rearrange("(t p u) -> t p u", p=128, u=U)   # [4,128,16]

    def normalize_group(g):
        pst = pin.tile([128, U * D], F32, name="pst", tag="pst", bufs=2)
        nc.sync.dma_start(out=pst, in_=ps4[g])
        sq = work.tile([128, U * D], BF16, name="sq", tag="sq", bufs=2)
        nc.vector.tensor_mul(sq, pst, pst)
        nc.vector.tensor_reduce(
            rn_f[:, g * U : (g + 1) * U],
            sq.rearrange("p (u d) -> p u d", u=U),
            axis=mybir.AxisListType.X,
            op=ALU.add,
        )
        _newton_rsqrt(
            nc, work, rn_f[:, g * U : (g + 1) * U], rn_f[:, g * U : (g + 1) * U]
        )
        nc.sync.dma_start(
            out=rn_store[g],
            in_=rn_f[:, g * U : (g + 1) * U].rearrange("p (t u) -> p t u", u=U),
        )
        rnb = work.tile([128, GROUP], F32, name="rnb", tag="rnb", bufs=2)
        nc.sync.dma_start(
            out=rnb,
            in_=dram_rn[g * GROUP : (g + 1) * GROUP].partition_broadcast(128),
        )
        for k, qtk in enumerate((qt0, qt1)):
            ptc = pin.tile([128, GROUP], F32, name="ptc", tag="ptc", bufs=4)
            nc.gpsimd.dma_start(
                out=ptc,
                in_=pt[k * 128 : (k + 1) * 128, g * GROUP : (g + 1) * GROUP],
            )
            nc.vector.tensor_mul(
                qtk[:, g * GROUP : (g + 1) * GROUP], ptc, rnb
            )

    normalize_group(0)

    # ---- Main loop: S-block matmuls + fused exp/row-sum ----
    for g in range(N_GROUPS):
        if g + 1 < N_GROUPS:
            normalize_group(g + 1)
        for m in range(M_TILES):
            ps = psum_pool.tile([128, GROUP], F32, name="ps")
            for n4 in range(N_PER_GROUP):
                col = g * GROUP + n4 * N_COLS
                for k, (btk, qtk) in enumerate(((bt0, qt0), (bt1, qt1))):
                    nc.tensor.matmul(
                        ps[:, n4 * N_COLS : (n4 + 1) * N_COLS],
                        btk[:, m * 128 : (m + 1) * 128],
                        qtk[:, col : col + N_COLS],
                        start=(k == 0),
                        stop=(k == 1),
                    )
            esc = epool.tile([128, GROUP], BF16, name="esc")
            nc.scalar.activation(
                out=esc,
                in_=ps,
                func=AF.Exp,
                scale=2.0,
                accum_out=sums[:, g * M_TILES + m : g * M_TILES + m + 1],
            )

    # ---- Epilogue: rowsum over groups, drop diagonal, log, write out ----
    sums3 = sums.rearrange("p (g m) -> p m g", g=N_GROUPS)
    nc.vector.tensor_reduce(rowsum, sums3, axis=mybir.AxisListType.X, op=ALU.add)
    nc.scalar.activation(out=exps, in_=selfdot, func=AF.Exp, scale=2.0)
    nc.vector.tensor_tensor(lse, rowsum, exps, op=ALU.subtract)
    nc.scalar.activation(out=lse, in_=lse, func=AF.Ln)
    nc.sync.dma_start(out=lse_out, in_=lse)

    for p in (epool, psum_pool, dram, work, pin, persist):
        p.release()


_BUILT = None


def _build():
    global _BUILT
    if _BUILT is None:
        nc = bacc.Bacc("TRN2", target_bir_lowering=False, debug=False,
                       num_devices=N_CORES)
        p_stats = nc.dram_tensor("p_stats", [B, D], F32, kind="ExternalInput").ap()
        pt = nc.dram_tensor("pt", [D, B], F32, kind="ExternalInput").ap()
        eye = nc.dram_tensor("eye", [128, 128], BF16, kind="ExternalInput").ap()
        p_blk = nc.dram_tensor("p_blk", [BLK, D], F32, kind="ExternalInput").ap()
        lse_out = nc.dram_tensor("lse_out", [128, M_TILES], F32,
                                 kind="ExternalOutput").ap()
        with tile.TileContext(nc) as tc:
            _emit(tc, p_stats, pt, p_blk, eye, lse_out)
        nc.finalize()
        _BUILT = nc
    return _BUILT


def run_on_hw(P, **spmd_kwargs):
    import jax.numpy as jnp

    nc = _build()
    pt_f32 = np.ascontiguousarray(P.T)
    eye = np.asarray(jnp.eye(128, dtype=jnp.bfloat16))
    in_maps = [
        {
            "p_stats": P,
            "pt": pt_f32,
            "p_blk": np.ascontiguousarray(P[c * BLK : (c + 1) * BLK]),
            "eye": eye,
        }
        for c in range(N_CORES)
    ]
    return bass_utils.run_bass_kernel_spmd(
        nc, in_maps, core_ids=list(range(N_CORES)), **spmd_kwargs
    )


def kernel(embedding1, embedding2, projection1, projection2):
    import jax.numpy as jnp

    # embeddings are unused by the reference computation
    P = np.ascontiguousarray(
        np.concatenate([projection1, projection2], axis=0), dtype=np.float32
    )
    res = run_on_hw(P)
    # reassemble per-row lse: core c, tile column m, partition p ->
    # global row c*1024 + m*128 + p
    lse_rows = np.empty(B, np.float32)
    for c in range(N_CORES):
        arr = np.asarray(res.results[c]["lse_out"])  # [128, M_TILES]
        lse_rows[c * BLK : (c + 1) * BLK] = arr.T.reshape(-1)
    # Reference fp32 semantics: logp_ii = f32(-2e9 - lse_i) (== -2e9 for
    # any |lse| < 128), then loss = -mean(logp) with the platform's XLA
    # fp32 reduction -- reproduce it bit-for-bit.
    logp = (np.float32(-2.0e9) - lse_rows).astype(np.float32)
    loss = -jnp.mean(jnp.asarray(logp))
    return np.asarray(loss)



# revision 2
# speedup vs baseline: 1.2875x; 1.2875x over previous
"""Contrastive (SimCLR-style) loss on 8 Trainium2 NeuronCores.

Math (matches the reference exactly):
  P = concat(projection1, projection2)            # [8192, 256]
  sim = cos_sim(P_i, P_j); diag masked to -1e9; logits = sim / 0.5
  labels = arange(2B)  -> picks the masked diagonal, so
  loss = -mean_i( logp_ii ),  logp_ii = f32(-2e9 - lse_i),
  lse_i = log(sum_{j != i} exp(2*sim_ij))

Distribution: data-parallel over the 8192 rows, one 1024-row block per
core.  Each core receives the projection matrix with its columns
ROTATED so that its own block occupies columns [0, 1024): this makes
the program identical across cores (SPMD) and, crucially, makes the
matmul lhsT tiles plain column slices of the already-normalized
transposed operand -- no on-chip transpose, no separate row-block
normalization path.

Per core, fully pipelined over four 2048-column groups:
  - stats load (row-major, 16-rows-per-partition interleave), square +
    reduce on VectorE, Newton-rsqrt (bit-trick seed, no ScalarE),
  - 1/norm -> DRAM -> partition-broadcast back, scale the bf16
    transposed operand -> normalized qt,
  - 8x [128 rows x 2048 cols] matmuls per group (bf16, fp32 PSUM),
    k-outer order so 4 consecutive matmuls share stationary weights,
  - exp evaluated IN-PLACE on the PSUM tile (ScalarE's fastest port)
    with fused row-sum accumulation.
Diagonal term: both operands are column-normalized so S_ii = 1 to
~1e-3; subtracting the constant e^2 from the row sum removes it with
O(2e-6) relative effect on lse.  Host applies the reference's fp32
arithmetic for the final mean.
"""

import sys

for _p in ("/opt/trn_rl_repo", "/root/.axon_site/_ro/trn_rl_repo"):
    if _p not in sys.path:
        sys.path.append(_p)

import numpy as np

import concourse.bacc as bacc
import concourse.tile as tile
from concourse import mybir
from concourse import bass_utils

F32 = mybir.dt.float32
BF16 = mybir.dt.bfloat16
I32 = mybir.dt.int32
AF = mybir.ActivationFunctionType
ALU = mybir.AluOpType

N_CORES = 8
B = 8192          # total rows (2 * batch)
D = 256           # projection dim
BLK = B // N_CORES        # 1024 rows per core
M_TILES = BLK // 128      # 8 row tiles per core
N_COLS = 512              # matmul free dim (one PSUM bank)
GROUP = 2048              # column group = 4 PSUM banks per exp tile
N_GROUPS = B // GROUP     # 4
N_PER_GROUP = GROUP // N_COLS  # 4
U = 16                    # consecutive rows per partition in stats loads
RSQRT_MAGIC = 0x5F3759DF
E_SQUARED = 7.38905609893065  # exp(2 * S_ii), S_ii == 1 after normalization


def _newton_rsqrt(nc, pool, out_rn, s):
    """out_rn = 1/sqrt(s), entirely on VectorE (fp32 internally).

    Quake-style bit seed + 2 Newton iterations (~5e-6 rel err).  Keeps
    ScalarE free for exp and avoids sqrt<->exp table reloads.
    """
    p, w = s.shape
    ibits = pool.tile([p, w], I32, name="ibits", tag="rsq_i", bufs=2)
    nc.vector.tensor_scalar(
        out=ibits, in0=s.bitcast(I32), scalar1=1, scalar2=None,
        op0=ALU.arith_shift_right,
    )
    nc.vector.tensor_scalar(
        out=ibits, in0=ibits, scalar1=-1, scalar2=RSQRT_MAGIC,
        op0=ALU.mult, op1=ALU.add,
    )
    y = ibits.bitcast(F32)
    t1 = pool.tile([p, w], F32, name="t1", tag="rsq_t1", bufs=2)
    for _ in range(2):
        nc.vector.tensor_mul(t1, y, y)
        nc.vector.tensor_mul(t1, t1, s)
        nc.vector.tensor_scalar(
            out=t1, in0=t1, scalar1=-0.5, scalar2=1.5,
            op0=ALU.mult, op1=ALU.add,
        )
        nc.vector.tensor_mul(y, y, t1)
    nc.vector.tensor_copy(out_rn, y)


def _emit(tc, psb, ptb, lse_out):
    nc = tc.nc

    persist = tc.alloc_tile_pool(name="persist", bufs=1)
    pin = tc.alloc_tile_pool(name="pin", bufs=2)
    work = tc.alloc_tile_pool(name="work", bufs=2)
    dram = tc.alloc_tile_pool(name="dram", bufs=1, space="DRAM")
    psum_pool = tc.alloc_tile_pool(name="psum", bufs=2, space="PSUM")

    # Persistent tensors: the normalized transposed operand (both k-halves)
    qt0 = persist.tile([128, B], BF16, tag="qt0", name="qt0")
    qt1 = persist.tile([128, B], BF16, tag="qt1", name="qt1")
    sums = persist.tile([128, N_GROUPS * M_TILES], F32, tag="sums", name="sums")
    rowsum = persist.tile([128, M_TILES], F32, tag="rowsum", name="rowsum")
    lse = persist.tile([128, M_TILES], F32, tag="lse", name="lse")
    dram_rn = dram.tile([B], BF16, tag="dram_rn", name="dram_rn")

    # stats load: row j = 2048g + 16p + u  ->  group g, partition p, slot u
    ps_il = psb.rearrange("(g p u) d -> g p (u d)", p=128, u=U)   # [4,128,4096]
    # rn store: dram_rn[2048g + 16p + u] <- rn16[p, u]
    rn_store = dram_rn.rearrange("(g p u) -> g p u", p=128, u=U)  # [4,128,16]

    def normalize_group(g):
        # row norms for columns [2048g, 2048(g+1))
        pst = pin.tile([128, U * D], BF16, name="pst", tag="pst", bufs=2)
        nc.gpsimd.dma_start(out=pst, in_=ps_il[g])
        sq = work.tile([128, U * D], BF16, name="sq", tag="sq", bufs=2)
        nc.vector.tensor_mul(sq, pst, pst)
        s = work.tile([128, U], F32, name="s", tag="s", bufs=2)
        nc.vector.tensor_reduce(
            s, sq.rearrange("p (u d) -> p u d", u=U),
            axis=mybir.AxisListType.X, op=ALU.add,
        )
        rn16 = work.tile([128, U], BF16, name="rn16", tag="rn16", bufs=2)
        _newton_rsqrt(nc, work, rn16, s)
        nc.sync.dma_start(out=rn_store[g], in_=rn16)
        rnb = pin.tile([128, GROUP], BF16, name="rnb", tag="rnb", bufs=2)
        nc.sync.dma_start(
            out=rnb,
            in_=dram_rn[g * GROUP : (g + 1) * GROUP].partition_broadcast(128),
        )
        # qt[:, group] = ptb[:, group] * (1/norm), both k-halves
        for k, qtk in enumerate((qt0, qt1)):
            ptc = pin.tile([128, GROUP], BF16, name="ptc", tag="ptc", bufs=4)
            nc.gpsimd.dma_start(
                out=ptc,
                in_=ptb[k * 128 : (k + 1) * 128, g * GROUP : (g + 1) * GROUP],
            )
            nc.vector.tensor_mul(qtk[:, g * GROUP : (g + 1) * GROUP], ptc, rnb)

    normalize_group(0)

    # ---- Main loop: S-block matmuls + fused in-place exp/row-sum ----
    for g in range(N_GROUPS):
        if g + 1 < N_GROUPS:
            normalize_group(g + 1)
        for m in range(M_TILES):
            ps = psum_pool.tile([128, GROUP], F32, name="ps")
            # k-outer: 4 consecutive matmuls share the same stationary tile
            for k, qtk in enumerate((qt0, qt1)):
                for n4 in range(N_PER_GROUP):
                    col = g * GROUP + n4 * N_COLS
                    nc.tensor.matmul(
                        ps[:, n4 * N_COLS : (n4 + 1) * N_COLS],
                        qtk[:, m * 128 : (m + 1) * 128],
                        qtk[:, col : col + N_COLS],
                        start=(k == 0),
                        stop=(k == 1),
                    )
            nc.scalar.activation(
                out=ps,
                in_=ps,
                func=AF.Exp,
                scale=2.0,
                accum_out=sums[:, g * M_TILES + m : g * M_TILES + m + 1],
            )

    # ---- Epilogue: rowsum over groups, drop diagonal, log, write out ----
    sums3 = sums.rearrange("p (g m) -> p m g", g=N_GROUPS)
    nc.vector.tensor_reduce(rowsum, sums3, axis=mybir.AxisListType.X, op=ALU.add)
    nc.vector.tensor_scalar(
        out=rowsum, in0=rowsum, scalar1=-E_SQUARED, scalar2=None, op0=ALU.add,
    )
    nc.scalar.activation(out=lse, in_=rowsum, func=AF.Ln)
    nc.sync.dma_start(out=lse_out, in_=lse)

    for p in (psum_pool, dram, work, pin, persist):
        p.release()


_BUILT = None


def _build():
    global _BUILT
    if _BUILT is None:
        nc = bacc.Bacc("TRN2", target_bir_lowering=False, debug=False,
                       num_devices=N_CORES)
        psb = nc.dram_tensor("psb", [B, D], BF16, kind="ExternalInput").ap()
        ptb = nc.dram_tensor("ptb", [D, B], BF16, kind="ExternalInput").ap()
        lse_out = nc.dram_tensor("lse_out", [128, M_TILES], F32,
                                 kind="ExternalOutput").ap()
        with tile.TileContext(nc) as tc:
            _emit(tc, psb, ptb, lse_out)
        nc.finalize()
        _BUILT = nc
    return _BUILT


def run_on_hw(P, **spmd_kwargs):
    import jax.numpy as jnp

    nc = _build()
    # one bf16 conversion, then cheap per-core rolls
    Pb = np.asarray(jnp.asarray(P, jnp.bfloat16))          # [8192, 256] bf16
    Ptb = np.ascontiguousarray(Pb.T)                       # [256, 8192] bf16
    in_maps = []
    for c in range(N_CORES):
        off = c * BLK
        in_maps.append({
            "psb": np.ascontiguousarray(np.roll(Pb, -off, axis=0)),
            "ptb": np.ascontiguousarray(np.roll(Ptb, -off, axis=1)),
        })
    return bass_utils.run_bass_kernel_spmd(
        nc, in_maps, core_ids=list(range(N_CORES)), **spmd_kwargs
    )


def lse_rows_from_results(res):
    """Per-row logsumexp, reassembled: core c, tile column m, partition p
    -> global row c*1024 + m*128 + p."""
    lse_rows = np.empty(B, np.float32)
    for c in range(N_CORES):
        arr = np.asarray(res.results[c]["lse_out"])  # [128, M_TILES]
        lse_rows[c * BLK : (c + 1) * BLK] = arr.T.reshape(-1)
    return lse_rows


def kernel(embedding1, embedding2, projection1, projection2):
    import jax.numpy as jnp

    # embeddings are unused by the reference computation
    P = np.ascontiguousarray(
        np.concatenate([projection1, projection2], axis=0), dtype=np.float32
    )
    res = run_on_hw(P)
    lse_rows = lse_rows_from_results(res)
    # Reference fp32 semantics: logp_ii = f32(-2e9 - lse_i) (== -2e9 for
    # any |lse| < 128), then loss = -mean(logp) with the platform's XLA
    # fp32 reduction -- reproduce it bit-for-bit.
    logp = (np.float32(-2.0e9) - lse_rows).astype(np.float32)
    loss = -jnp.mean(jnp.asarray(logp))
    return np.asarray(loss)
